# revision 5
# baseline (speedup 1.0000x reference)
"""Trainium2 Bass kernel: LSTM decoder with Luong attention + greedy feedback.

Sharding over 8 cores:
  - LSTM gate matmul: output(gate)-sharded (each core computes gates[:, 512c:512c+512])
  - Attention: batch-sharded (core c owns batch rows 4c:4c+4)
  - fc [2048, 32000]: vocab-sharded (core c owns cols 4000c:4000c+4000), weights
    streamed from HBM every step (262MB total fp32 exceeds aggregate SBUF)
  - 3 small AllGathers per step: gates, ctx^T, (max, argmax) stats
  - preact written to DRAM in-loop; tanh + global log-softmax done in a
    parallel phase 2 afterwards.
"""
import sys
sys.path.insert(0, "/opt/trn_rl_repo")
import numpy as np
import concourse.bass as bass
import concourse.tile as tile
from concourse import bacc, mybir
from concourse import bass_utils
from concourse.masks import make_identity

F32 = mybir.dt.float32
U32 = mybir.dt.uint32
I32 = mybir.dt.int32
AX = mybir.AxisListType.X
Act = mybir.ActivationFunctionType

NCORES = 8
B, U, E, V, S, T = 32, 1024, 512, 32000, 128, 64
BL = B // NCORES          # 4 batch rows per core (attention shard)
GL = 4 * U // NCORES      # 512 gate outputs per core
VL = V // NCORES          # 4000 vocab per core
NCH = 500                 # fc n-chunk (PSUM bank = 512 f32)
NNC = VL // NCH           # 8 n-chunks
KC, KE, KU = 16, E // 128, U // 128   # fc k-chunks, emb k-chunks(4), U k-chunks(8)
ROWS = T * B              # 2048 preact rows
RCH = ROWS // 128         # 16 phase-2 chunks
START_TOK = 2


def build_nc(t_steps=T):
    nc = bacc.Bacc("TRN2", target_bir_lowering=False, debug=False, num_devices=NCORES)
    dt = nc.dram_tensor
    emb_d = dt("emb", [V, E], F32, kind="ExternalInput")
    enc_s_d = dt("enc_s", [128, BL * U], F32, kind="ExternalInput")
    encT_d = dt("encT", [128, BL * U], F32, kind="ExternalInput")
    WaT_d = dt("WaT", [128, KU * U], F32, kind="ExternalInput")
    Wk_d = dt("Wk", [128, KE * GL], F32, kind="ExternalInput")
    Wr_d = dt("Wr", [128, KU * GL], F32, kind="ExternalInput")
    lb_d = dt("lb", [1, GL], F32, kind="ExternalInput")
    fcw_d = dt("fcw", [KC, 128, VL], F32, kind="ExternalInput")
    fcb_d = dt("fcb", [1, VL], F32, kind="ExternalInput")
    h0T_d = dt("h0T", [128, KU * B], F32, kind="ExternalInput")
    c0_d = dt("c0", [B, U], F32, kind="ExternalInput")
    tok0_d = dt("tok0", [B, 1], U32, kind="ExternalInput")
    sel_d = dt("sel", [B, BL], F32, kind="ExternalInput")
    voff_d = dt("voff", [B, 1], F32, kind="ExternalInput")

    rows = t_steps * B
    preact_d = dt("preact", [rows, VL], F32)
    logp_d = dt("logp", [rows, VL], F32, kind="ExternalOutput")
    toks_d = dt("toks", [B, t_steps], I32, kind="ExternalOutput")

    with tile.TileContext(nc) as tc:
        with (
            tc.tile_pool(name="res", bufs=1) as res,       # persistent residents
            tc.tile_pool(name="st", bufs=1) as stp,        # persistent state
        ):
            # ---- residents ----
            ident = res.tile([128, 128], F32)
            make_identity(nc, ident[:])
            ones = res.tile([1, B], F32)
            nc.vector.memset(ones[:], 1.0)
            enc_s = res.tile([128, BL * U], F32)
            nc.sync.dma_start(enc_s[:], enc_s_d[:, :])
            encT = res.tile([128, BL * U], F32)
            nc.sync.dma_start(encT[:], encT_d[:, :])
            WaT = res.tile([128, KU * U], F32)
            nc.sync.dma_start(WaT[:], WaT_d[:, :])
            Wk = res.tile([128, KE * GL], F32)
            nc.sync.dma_start(Wk[:], Wk_d[:, :])
            Wr = res.tile([128, KU * GL], F32)
            nc.sync.dma_start(Wr[:], Wr_d[:, :])
            lb = res.tile([1, GL], F32)
            nc.sync.dma_start(lb[:], lb_d[:, :])
            sel = res.tile([B, BL], F32)
            nc.sync.dma_start(sel[:], sel_d[:, :])
            voff = res.tile([B, 1], F32)
            nc.sync.dma_start(voff[:], voff_d[:, :])

            # ---- state ----
            hT = stp.tile([128, KU * B], F32)       # h^T chunks [128, (k b)]
            nc.sync.dma_start(hT[:], h0T_d[:, :])
            cst = stp.tile([B, U], F32)
            nc.sync.dma_start(cst[:], c0_d[:, :])
            tok = stp.tile([B, 1], U32)
            nc.sync.dma_start(tok[:], tok0_d[:, :])
            toks_f = stp.tile([B, t_steps], F32)

            with (
                tc.tile_pool(name="wstream", bufs=2) as wsp,
                tc.tile_pool(name="work", bufs=1) as wk,
                tc.tile_pool(name="small", bufs=3) as sm,
                tc.tile_pool(name="psA", bufs=2, space="PSUM") as psA,   # [32,512]-ish
                tc.tile_pool(name="psT", bufs=3, space="PSUM") as psT,   # transposes [128,<=32]
                tc.tile_pool(name="psF", bufs=2, space="PSUM") as psF,   # fc [32,500]
                tc.tile_pool(name="dram", bufs=2, space="DRAM") as drp,
            ):
                for t in range(t_steps):
                    # 1. x = emb[tok]  -> xT chunks
                    x_sb = wk.tile([B, E], F32, tag="x")
                    nc.gpsimd.indirect_dma_start(
                        out=x_sb[:], out_offset=None, in_=emb_d[:, :],
                        in_offset=bass.IndirectOffsetOnAxis(ap=tok[:, :1], axis=0),
                    )
                    xT = wk.tile([128, KE * B], F32, tag="xT")
                    for k in range(KE):
                        pt = psT.tile([128, B], F32, tag="ptr")
                        nc.tensor.transpose(pt[:], x_sb[:, 128 * k:128 * (k + 1)], ident[:B, :B])
                        nc.vector.tensor_copy(xT[:, B * k:B * (k + 1)], pt[:])

                    # 2. gates slice [B, GL] = x@Wk_loc + h@Wr_loc + b_loc
                    pg = psA.tile([B, GL], F32, tag="pg")
                    for k in range(KE):
                        nc.tensor.matmul(pg[:], lhsT=xT[:, B * k:B * (k + 1)],
                                         rhs=Wk[:, GL * k:GL * (k + 1)],
                                         start=(k == 0), stop=False)
                    for k in range(KU):
                        nc.tensor.matmul(pg[:], lhsT=hT[:, B * k:B * (k + 1)],
                                         rhs=Wr[:, GL * k:GL * (k + 1)],
                                         start=False, stop=False)
                    nc.tensor.matmul(pg[:], lhsT=ones[:1, :], rhs=lb[:1, :],
                                     start=False, stop=True)
                    gpart = wk.tile([B, GL], F32, tag="gpart")
                    nc.vector.tensor_copy(gpart[:], pg[:])

                    # 3. AllGather gates -> [B, 4U]
                    ccg_i = drp.tile([B, GL], F32, tag="ccgi")
                    ccg_o = drp.tile([NCORES * B, GL], F32, tag="ccgo")
                    nc.sync.dma_start(ccg_i[:], gpart[:])
                    nc.gpsimd.collective_compute(
                        "AllGather", mybir.AluOpType.bypass,
                        ins=[ccg_i.opt()], outs=[ccg_o.opt()],
                        replica_groups=[list(range(NCORES))],
                    )
                    gates = wk.tile([B, 4 * U], F32, tag="gates")
                    nc.sync.dma_start(
                        gates[:].rearrange("b (c n) -> b c n", c=NCORES),
                        ccg_o[:].rearrange("(c b) n -> b c n", b=B),
                    )

                    # 4. LSTM pointwise (replicated, all B rows)
                    sif = wk.tile([B, 2 * U], F32, tag="sif")
                    nc.scalar.activation(sif[:], gates[:, 0:2 * U], Act.Sigmoid)
                    gt_t = wk.tile([B, U], F32, tag="gt")
                    nc.scalar.activation(gt_t[:], gates[:, 2 * U:3 * U], Act.Tanh)
                    o_s = wk.tile([B, U], F32, tag="os")
                    nc.scalar.activation(o_s[:], gates[:, 3 * U:4 * U], Act.Sigmoid)
                    t1 = wk.tile([B, U], F32, tag="t1")
                    nc.vector.tensor_mul(t1[:], sif[:, U:2 * U], cst[:])
                    t2 = wk.tile([B, U], F32, tag="t2")
                    nc.vector.tensor_mul(t2[:], sif[:, 0:U], gt_t[:])
                    nc.vector.tensor_add(cst[:], t1[:], t2[:])
                    tc_t = wk.tile([B, U], F32, tag="tct")
                    nc.scalar.activation(tc_t[:], cst[:], Act.Tanh)
                    h_sb = wk.tile([B, U], F32, tag="h")
                    nc.vector.tensor_mul(h_sb[:], o_s[:], tc_t[:])

                    # 5. hT (all B) + h_loc/hTBc (my 4 rows via selection matmul)
                    for k in range(KU):
                        pt = psT.tile([128, B], F32, tag="ptr")
                        nc.tensor.transpose(pt[:], h_sb[:, 128 * k:128 * (k + 1)], ident[:B, :B])
                        nc.vector.tensor_copy(hT[:, B * k:B * (k + 1)], pt[:])
                    hl = wk.tile([BL, U], F32, tag="hl")
                    for k in range(KU):
                        ph = psT.tile([BL, 128], F32, tag="ptr")
                        nc.tensor.matmul(ph[:], lhsT=sel[:, :],
                                         rhs=h_sb[:, 128 * k:128 * (k + 1)],
                                         start=True, stop=True)
                        nc.vector.tensor_copy(hl[:, 128 * k:128 * (k + 1)], ph[:])
                    hTB = wk.tile([128, KU * BL], F32, tag="hTB")
                    for k in range(KU):
                        pt = psT.tile([128, B], F32, tag="ptr")
                        nc.tensor.transpose(pt[:, :BL], hl[:, 128 * k:128 * (k + 1)], ident[:BL, :BL])
                        nc.vector.tensor_copy(hTB[:, BL * k:BL * (k + 1)], pt[:, :BL])

                    # 6. qq = h_loc @ Wa^T   [BL, U], then qqT [128, (k bl)]
                    qq = wk.tile([BL, U], F32, tag="qq")
                    for h2 in range(2):
                        pq = psA.tile([BL, 512], F32, tag="pg")
                        for k in range(KU):
                            nc.tensor.matmul(pq[:], lhsT=hTB[:, BL * k:BL * (k + 1)],
                                             rhs=WaT[:, U * k + 512 * h2:U * k + 512 * (h2 + 1)],
                                             start=(k == 0), stop=(k == KU - 1))
                        nc.vector.tensor_copy(qq[:, 512 * h2:512 * (h2 + 1)], pq[:])
                    qqT = wk.tile([128, KU * BL], F32, tag="qqT")
                    for k in range(KU):
                        pt = psT.tile([128, B], F32, tag="ptr")
                        nc.tensor.transpose(pt[:, :BL], qq[:, 128 * k:128 * (k + 1)], ident[:BL, :BL])
                        nc.vector.tensor_copy(qqT[:, BL * k:BL * (k + 1)], pt[:, :BL])

                    # 7. scores [1, S] per local row; per-row softmax -> attnT [128, BL]
                    attnT = sm.tile([128, BL], F32, tag="attnT")
                    for bl in range(BL):
                        ps = psT.tile([1, S], F32, tag="ptr")
                        for k in range(KU):
                            nc.tensor.matmul(
                                ps[:], lhsT=qqT[:, BL * k + bl:BL * k + bl + 1],
                                rhs=encT[:, U * bl + S * k:U * bl + S * (k + 1)],
                                start=(k == 0), stop=(k == KU - 1))
                        sc = sm.tile([1, S], F32, tag="sc")
                        nc.vector.tensor_copy(sc[:], ps[:])
                        nm = sm.tile([1, 1], F32, tag="nm")
                        nc.vector.reduce_max(nm[:], sc[:], axis=AX, negate=True)
                        esc = sm.tile([1, S], F32, tag="esc")
                        nc.scalar.activation(esc[:], sc[:], Act.Exp, bias=nm[:, 0:1])
                        ssum = sm.tile([1, 1], F32, tag="ssum")
                        nc.vector.reduce_sum(ssum[:], esc[:], axis=AX)
                        rs = sm.tile([1, 1], F32, tag="rs")
                        nc.vector.reciprocal(rs[:], ssum[:])
                        attn = sm.tile([1, S], F32, tag="attn")
                        nc.vector.tensor_scalar_mul(attn[:], esc[:], rs[:, 0:1])
                        paT = psT.tile([128, B], F32, tag="ptr")
                        nc.tensor.transpose(paT[:, :1], attn[:], ident[:1, :1])
                        nc.vector.tensor_copy(attnT[:, bl:bl + 1], paT[:, :1])
                    # 8. ctxT [128, (k bl)] = attn-weighted sum of enc
                    ctxT = wk.tile([128, KU * BL], F32, tag="ctxT")
                    for m in range(KU):
                        pc = psT.tile([128, BL], F32, tag="ptr")
                        for bl in range(BL):
                            nc.tensor.matmul(
                                pc[:, bl:bl + 1],
                                lhsT=enc_s[:, U * bl + 128 * m:U * bl + 128 * (m + 1)],
                                rhs=attnT[:, bl:bl + 1],
                                start=True, stop=True)
                        nc.vector.tensor_copy(ctxT[:, BL * m:BL * (m + 1)], pc[:])

                    # 9. AllGather ctxT -> aT_ctx [128, (k b)] with b=(c, bl)
                    ccc_i = drp.tile([128, KU * BL], F32, tag="ccci")
                    ccc_o = drp.tile([NCORES * 128, KU * BL], F32, tag="ccco")
                    nc.sync.dma_start(ccc_i[:], ctxT[:])
                    nc.gpsimd.collective_compute(
                        "AllGather", mybir.AluOpType.bypass,
                        ins=[ccc_i.opt()], outs=[ccc_o.opt()],
                        replica_groups=[list(range(NCORES))],
                    )
                    aTc = wk.tile([128, KU * B], F32, tag="aTc")
                    nc.sync.dma_start(
                        aTc[:].rearrange("p (k c bl) -> p k c bl", c=NCORES, bl=BL),
                        ccc_o[:].rearrange("(c p) (k bl) -> p k c bl", p=128, bl=BL),
                    )

                    # 10. fc: preact [B, VL] = [ctx, h] @ fcW_loc + fcb_loc
                    # (W streamed from DRAM in two half-k tiles per n-chunk)
                    mall = sm.tile([B, 8], F32, tag="mall")
                    iallf = sm.tile([B, 8], F32, tag="iallf")
                    for n in range(NNC):
                        sl = slice(NCH * n, NCH * (n + 1))
                        wtA = wsp.tile([128, KU * NCH], F32, tag="wt")
                        nc.sync.dma_start(
                            wtA[:].rearrange("p (k n) -> p k n", k=KU),
                            fcw_d[0:KU, :, sl].rearrange("k p n -> p k n"),
                        )
                        wtB = wsp.tile([128, KU * NCH], F32, tag="wt")
                        nc.sync.dma_start(
                            wtB[:].rearrange("p (k n) -> p k n", k=KU),
                            fcw_d[KU:KC, :, sl].rearrange("k p n -> p k n"),
                        )
                        fcbt = sm.tile([1, NCH], F32, tag="fcbt")
                        nc.sync.dma_start(fcbt[:], fcb_d[0:1, sl])
                        pf = psF.tile([B, NCH], F32, tag="pf")
                        for k in range(KU):
                            nc.tensor.matmul(pf[:], lhsT=hT[:, B * k:B * (k + 1)],
                                             rhs=wtB[:, NCH * k:NCH * (k + 1)],
                                             start=(k == 0), stop=False)
                        for k in range(KU):
                            nc.tensor.matmul(pf[:], lhsT=aTc[:, B * k:B * (k + 1)],
                                             rhs=wtA[:, NCH * k:NCH * (k + 1)],
                                             start=False, stop=False)
                        nc.tensor.matmul(pf[:], lhsT=ones[:1, :], rhs=fcbt[:1, :],
                                         start=False, stop=True)
                        prec = wk.tile([B, NCH], F32, tag="prec")
                        nc.vector.tensor_copy(prec[:], pf[:])
                        nc.sync.dma_start(preact_d[B * t:B * (t + 1), sl], prec[:])
                        m8c = sm.tile([B, 8], F32, tag="m8c")
                        i8c = sm.tile([B, 8], U32, tag="i8c")
                        nc.vector.max_with_indices(m8c[:], i8c[:], prec[:])
                        nc.vector.tensor_copy(mall[:, n:n + 1], m8c[:, 0:1])
                        nc.vector.tensor_copy(iallf[:, n:n + 1], i8c[:, 0:1])
                    # 11. combine per-chunk maxima -> local (max, argmax)
                    m8f = sm.tile([B, 8], F32, tag="m8f")
                    i8f = sm.tile([B, 8], U32, tag="i8f")
                    nc.vector.max_with_indices(m8f[:], i8f[:], mall[:])
                    chf = sm.tile([B, 1], F32, tag="chf")
                    nc.vector.tensor_copy(chf[:], i8f[:, 0:1])
                    iloc = sm.tile([B, 1], F32, tag="iloc")
                    nc.vector.memset(iloc[:], 0.0)
                    for n in range(NNC):
                        eqn = sm.tile([B, 1], F32, tag="eqn")
                        nc.vector.tensor_scalar(out=eqn[:], in0=chf[:], scalar1=float(n),
                                                scalar2=None, op0=mybir.AluOpType.is_equal)
                        nc.vector.tensor_mul(eqn[:], eqn[:], iallf[:, n:n + 1])
                        nc.vector.tensor_add(iloc[:], iloc[:], eqn[:])
                    st2 = sm.tile([B, 2], F32, tag="st2")
                    nc.vector.tensor_copy(st2[:, 0:1], m8f[:, 0:1])
                    nc.vector.tensor_scalar(out=st2[:, 1:2], in0=chf[:], scalar1=float(NCH),
                                            scalar2=None, op0=mybir.AluOpType.mult)
                    nc.vector.tensor_add(st2[:, 1:2], st2[:, 1:2], iloc[:])
                    nc.vector.tensor_add(st2[:, 1:2], st2[:, 1:2], voff[:])
                    ccs_i = drp.tile([B, 2], F32, tag="ccsi")
                    ccs_o = drp.tile([NCORES * B, 2], F32, tag="ccso")
                    nc.sync.dma_start(ccs_i[:], st2[:])
                    nc.gpsimd.collective_compute(
                        "AllGather", mybir.AluOpType.bypass,
                        ins=[ccs_i.opt()], outs=[ccs_o.opt()],
                        replica_groups=[list(range(NCORES))],
                    )
                    sg = sm.tile([B, NCORES * 2], F32, tag="sg")
                    nc.sync.dma_start(
                        sg[:].rearrange("b (c two) -> b c two", two=2),
                        ccs_o[:].rearrange("(c b) two -> b c two", b=B),
                    )
                    bm = sm.tile([B, 1], F32, tag="bm")
                    bi = sm.tile([B, 1], F32, tag="bi")
                    nc.vector.tensor_copy(bm[:], sg[:, 0:1])
                    nc.vector.tensor_copy(bi[:], sg[:, 1:2])
                    for ci in range(1, NCORES):
                        mc = sg[:, 2 * ci:2 * ci + 1]
                        icf = sg[:, 2 * ci + 1:2 * ci + 2]
                        gtm = sm.tile([B, 1], F32, tag="gtm")
                        nc.vector.tensor_tensor(out=gtm[:], in0=mc, in1=bm[:],
                                                op=mybir.AluOpType.is_gt)
                        dd = sm.tile([B, 1], F32, tag="dd")
                        nc.vector.tensor_tensor(out=dd[:], in0=icf, in1=bi[:],
                                                op=mybir.AluOpType.subtract)
                        nc.vector.tensor_tensor(out=dd[:], in0=dd[:], in1=gtm[:],
                                                op=mybir.AluOpType.mult)
                        nc.vector.tensor_tensor(out=bi[:], in0=bi[:], in1=dd[:],
                                                op=mybir.AluOpType.add)
                        nc.vector.tensor_tensor(out=bm[:], in0=bm[:], in1=mc,
                                                op=mybir.AluOpType.max)
                    nc.vector.tensor_copy(toks_f[:, t:t + 1], bi[:])
                    nc.vector.tensor_copy(tok[:], bi[:])  # f32 -> u32 cast

                # dump tokens
                toks_i = stp.tile([B, t_steps], I32)
                nc.vector.tensor_copy(toks_i[:], toks_f[:])
                nc.sync.dma_start(toks_d[:, :], toks_i[:])

            # ---------- phase 2: tanh + global log-softmax ----------
            crw = min(128, rows)
            rch = rows // crw
            with (
                tc.tile_pool(name="p2", bufs=2) as p2,
                tc.tile_pool(name="p2s", bufs=1) as p2s,
                tc.tile_pool(name="dram2", bufs=1, space="DRAM") as dr2,
            ):
                mst = p2s.tile([crw, rch], F32)
                sst = p2s.tile([crw, rch], F32)
                for ch in range(rch):
                    pa = p2.tile([crw, VL], F32, tag="pa")
                    nc.sync.dma_start(pa[:], preact_d[crw * ch:crw * (ch + 1), :])
                    th = p2.tile([crw, VL], F32, tag="th")
                    nc.scalar.activation(th[:], pa[:], Act.Tanh)
                    nmx = p2.tile([crw, 1], F32, tag="nmx")
                    nc.vector.reduce_max(nmx[:], th[:], axis=AX, negate=True)
                    nc.scalar.activation(pa[:], th[:], Act.Exp, bias=nmx[:, 0:1])
                    nc.vector.reduce_sum(sst[:, ch:ch + 1], pa[:], axis=AX)
                    nc.vector.tensor_scalar_mul(mst[:, ch:ch + 1], nmx[:], -1.0)
                pk = p2s.tile([crw, 2 * rch], F32)
                nc.vector.tensor_copy(pk[:, 0:rch], mst[:])
                nc.vector.tensor_copy(pk[:, rch:2 * rch], sst[:])
                cc4_i = dr2.tile([crw, 2 * rch], F32)
                cc4_o = dr2.tile([NCORES * crw, 2 * rch], F32)
                nc.sync.dma_start(cc4_i[:], pk[:])
                nc.gpsimd.collective_compute(
                    "AllGather", mybir.AluOpType.bypass,
                    ins=[cc4_i.opt()], outs=[cc4_o.opt()],
                    replica_groups=[list(range(NCORES))],
                )
                g4 = p2s.tile([crw, NCORES * 2 * rch], F32)
                nc.sync.dma_start(
                    g4[:].rearrange("p (c f) -> p c f", c=NCORES),
                    cc4_o[:].rearrange("(c p) f -> p c f", p=crw),
                )
                W2 = 2 * rch
                mg = p2s.tile([crw, rch], F32)
                nc.vector.tensor_copy(mg[:], g4[:, 0:rch])
                for ci in range(1, NCORES):
                    nc.vector.tensor_tensor(out=mg[:], in0=mg[:],
                                            in1=g4[:, W2 * ci:W2 * ci + rch],
                                            op=mybir.AluOpType.max)
                sgs = p2s.tile([crw, rch], F32)
                nc.vector.memset(sgs[:], 0.0)
                for ci in range(NCORES):
                    d2 = p2.tile([crw, rch], F32, tag="d2")
                    nc.vector.tensor_tensor(out=d2[:], in0=g4[:, W2 * ci:W2 * ci + rch],
                                            in1=mg[:], op=mybir.AluOpType.subtract)
                    nc.scalar.activation(d2[:], d2[:], Act.Exp)
                    nc.vector.tensor_tensor(out=d2[:], in0=d2[:],
                                            in1=g4[:, W2 * ci + rch:W2 * ci + 2 * rch],
                                            op=mybir.AluOpType.mult)
                    nc.vector.tensor_tensor(out=sgs[:], in0=sgs[:], in1=d2[:],
                                            op=mybir.AluOpType.add)
                Kc = p2s.tile([crw, rch], F32)
                nc.scalar.activation(Kc[:], sgs[:], Act.Ln)
                nc.vector.tensor_tensor(out=Kc[:], in0=Kc[:], in1=mg[:],
                                        op=mybir.AluOpType.add)
                for ch in range(rch):
                    pa = p2.tile([crw, VL], F32, tag="pa")
                    nc.sync.dma_start(pa[:], preact_d[crw * ch:crw * (ch + 1), :])
                    th = p2.tile([crw, VL], F32, tag="th")
                    nc.scalar.activation(th[:], pa[:], Act.Tanh)
                    nc.vector.tensor_scalar_sub(pa[:], th[:], Kc[:, ch:ch + 1])
                    nc.sync.dma_start(logp_d[crw * ch:crw * (ch + 1), :], pa[:])
    nc.compile()
    return nc


_NC_CACHE = {}


def kernel(encoder_hidden, encoder_c, target, encoder_outputs, emb,
           lstm_kernel, lstm_rec, lstm_bias, Wa, Wa_b, fc_W, fc_b):
    encoder_hidden = np.asarray(encoder_hidden, np.float32)
    encoder_c = np.asarray(encoder_c, np.float32)
    enc = np.asarray(encoder_outputs, np.float32)
    emb = np.asarray(emb, np.float32)
    lstm_kernel = np.asarray(lstm_kernel, np.float32)
    lstm_rec = np.asarray(lstm_rec, np.float32)
    lstm_bias = np.asarray(lstm_bias, np.float32)
    Wa = np.asarray(Wa, np.float32)
    fc_W = np.asarray(fc_W, np.float32)
    fc_b = np.asarray(fc_b, np.float32)

    if "nc" not in _NC_CACHE:
        _NC_CACHE["nc"] = build_nc(T)
    nc = _NC_CACHE["nc"]

    WaT_full = np.ascontiguousarray(Wa.T)  # WaT[v, u] = Wa[u, v]
    in_maps = []
    for c in range(NCORES):
        Bc = slice(BL * c, BL * (c + 1))
        gl = slice(GL * c, GL * (c + 1))
        Vc = slice(VL * c, VL * (c + 1))
        ebc = enc[Bc]  # [BL, S, U]
        selm = np.zeros((B, BL), np.float32)
        for j in range(BL):
            selm[BL * c + j, j] = 1.0
        in_maps.append({
            "emb": emb,
            "enc_s": np.ascontiguousarray(ebc.transpose(1, 0, 2)).reshape(128, BL * U),
            "encT": np.ascontiguousarray(
                ebc.transpose(2, 0, 1).reshape(KU, 128, BL, S).transpose(1, 2, 0, 3)
            ).reshape(128, BL * U),
            "WaT": np.ascontiguousarray(
                WaT_full.reshape(KU, 128, U).transpose(1, 0, 2)).reshape(128, KU * U),
            "Wk": np.ascontiguousarray(
                lstm_kernel[:, gl].reshape(KE, 128, GL).transpose(1, 0, 2)
            ).reshape(128, KE * GL),
            "Wr": np.ascontiguousarray(
                lstm_rec[:, gl].reshape(KU, 128, GL).transpose(1, 0, 2)
            ).reshape(128, KU * GL),
            "lb": lstm_bias[gl].reshape(1, GL),
            "fcw": np.ascontiguousarray(fc_W[:, Vc].reshape(KC, 128, VL)),
            "fcb": fc_b[Vc].reshape(1, VL),
            "h0T": np.ascontiguousarray(
                encoder_hidden.T.reshape(KU, 128, B).transpose(1, 0, 2)
            ).reshape(128, KU * B),
            "c0": encoder_c,
            "tok0": np.full((B, 1), START_TOK, np.uint32),
            "sel": selm,
            "voff": np.full((B, 1), float(VL * c), np.float32),
        })
    res = bass_utils.run_bass_kernel_spmd(nc, in_maps, core_ids=list(range(NCORES)))
    shards = [r["logp"].reshape(T, B, VL) for r in res.results]
    logp = np.concatenate(shards, axis=-1)          # [T, B, V]
    decoder_output = np.ascontiguousarray(logp.transpose(1, 0, 2))  # [B, T, V]
    pred = np.ascontiguousarray(res.results[0]["toks"]).astype(np.int32)  # [B, T]
    return pred, decoder_output
